# revision 19
# baseline (speedup 1.0000x reference)
"""BertBiAttention Trainium2 kernel.

Cross-attention between two streams (B=4, S=2048, HID=768, H=12 heads).
Sharding: 8 cores = (stream s in {1,2}) x (batch b in {0..3}). Each core
computes one stream's full output for one batch element:
    h_s[b] = LayerNorm( attend(q_other, k_own, v_own, mask_own) @ wd + bd + x_own )
No collectives needed; the host stacks per-core outputs.

On-chip layouts (per core):
  qT, kT  [768, 2048] bf16  (feature-major, head h at partition rows h*64..)
  v       [2048 (16x128), 12, 65] bf16  (per head: [v*emask | emask] columns;
          odd heads store [emask | v*emask] so the PSUM partition ranges of
          the normalization never cross the 64-lane boundary)
  scoresT [krows, q] in PSUM -> exp (ACT, scale=1/8) -> bf16
  ctxT    accumulated via lhsT=[v|1] matmuls; row 64 (or 63) = softmax denom
  dense   h = ctxT.T @ wd (+bd via K=1 ones matmul) + residual, LayerNorm.
All matmuls fp32r (full inputs) or bf16 (attention path); PSUM accum fp32.
"""

import numpy as np

import concourse.bass as bass
import concourse.mybir as mybir
import concourse.tile as tile
from concourse import bacc, bass_utils
from concourse.masks import make_identity

B, S, HID, H, HD = 4, 2048, 768, 12, 64
FT = HID // 128   # 6 feature tiles
ST = S // 128     # 16 seq tiles
QT = S // 512     # 4 q chunks
NH = 2            # 768-wide outputs split into 2 x 384
NW = 384
EPS = 1e-12

F32 = mybir.dt.float32
F32R = mybir.dt.float32r
BF16 = mybir.dt.bfloat16
AF = mybir.ActivationFunctionType


def _bcast_part(ap, p=128):
    """DRAM row [1, N] -> partition-broadcast AP [p, N] (stride-0 partition)."""
    return bass.AP(tensor=ap.tensor, offset=ap.offset, ap=[[0, p], ap.ap[-1]])


def build_nc():
    nc = bacc.Bacc("TRN2", target_bir_lowering=False, debug=False, num_devices=8)

    xq_d = nc.dram_tensor("xq", [S, HID], F32, kind="ExternalInput").ap()
    xkv_d = nc.dram_tensor("xkv", [S, HID], F32, kind="ExternalInput").ap()
    wq_d = nc.dram_tensor("wq", [HID, HID], F32, kind="ExternalInput").ap()
    wk_d = nc.dram_tensor("wk", [HID, HID], F32, kind="ExternalInput").ap()
    wv_d = nc.dram_tensor("wv", [HID, HID], F32, kind="ExternalInput").ap()
    wd_d = nc.dram_tensor("wd", [HID, HID], F32, kind="ExternalInput").ap()
    bq_d = nc.dram_tensor("bq", [1, HID], F32, kind="ExternalInput").ap()
    bk_d = nc.dram_tensor("bk", [1, HID], F32, kind="ExternalInput").ap()
    bv_d = nc.dram_tensor("bv", [1, HID], F32, kind="ExternalInput").ap()
    bd_d = nc.dram_tensor("bd", [1, HID], F32, kind="ExternalInput").ap()
    mask_d = nc.dram_tensor("mask", [S, 1], F32, kind="ExternalInput").ap()
    lng_d = nc.dram_tensor("lng", [1, HID], F32, kind="ExternalInput").ap()
    lnb_d = nc.dram_tensor("lnb", [1, HID], F32, kind="ExternalInput").ap()
    out_d = nc.dram_tensor("out", [S, HID], F32, kind="ExternalOutput").ap()

    with tile.TileContext(nc) as tc:
        with (
            tc.tile_pool(name="consts", bufs=1) as consts,
            tc.tile_pool(name="big", bufs=1) as big,
        ):
            # ---- constants ----
            ident = consts.tile([128, 128], F32)
            make_identity(nc, ident)
            ones_r = consts.tile([1, 128], BF16)
            nc.vector.memset(ones_r, 1.0)
            ones_12 = consts.tile([128, 12], F32)
            nc.vector.memset(ones_12, 1.0)
            # ones at partition base 64 (lhsT for the denom-broadcast matmul,
            # whose K=1 contraction row must match rhs's base partition 64)
            ones2 = consts.tile([128, HD], BF16)
            nc.vector.memset(ones2, 1.0)
            eps_t = consts.tile([128, 1], F32)
            nc.vector.memset(eps_t, EPS)

            bqc = consts.tile([128, FT], F32)
            bkc = consts.tile([128, FT], F32)
            for f in range(FT):
                nc.sync.dma_start(
                    out=bqc[:, f : f + 1],
                    in_=bq_d[0:1, f * 128 : (f + 1) * 128].rearrange("a b -> b a"),
                )
                nc.sync.dma_start(
                    out=bkc[:, f : f + 1],
                    in_=bk_d[0:1, f * 128 : (f + 1) * 128].rearrange("a b -> b a"),
                )
            bv_f = consts.tile([1, HID], F32)
            nc.sync.dma_start(out=bv_f, in_=bv_d)
            bd_f = consts.tile([1, HID], F32)
            nc.sync.dma_start(out=bd_f, in_=bd_d)
            bv_row = consts.tile([1, HID], BF16)
            nc.vector.tensor_copy(out=bv_row, in_=bv_f)
            bd_row = consts.tile([1, HID], BF16)
            nc.vector.tensor_copy(out=bd_row, in_=bd_f)

            mask_t = consts.tile([128, ST], F32)
            for t in range(ST):
                nc.sync.dma_start(
                    out=mask_t[:, t : t + 1], in_=mask_d[t * 128 : (t + 1) * 128, :]
                )
            emask = consts.tile([128, ST], F32)
            nc.scalar.activation(out=emask, in_=mask_t, func=AF.Exp)

            # broadcast ln gamma/beta to all 128 partitions (stride-0 DMA)
            g_bc = consts.tile([128, HID], F32)
            b_bc = consts.tile([128, HID], F32)
            nc.sync.dma_start(out=g_bc, in_=_bcast_part(lng_d))
            nc.sync.dma_start(out=b_bc, in_=_bcast_part(lnb_d))

            # ---- persistent activation buffers ----
            qT = [big.tile([128, S], BF16, name=f"qT{f}") for f in range(FT)]
            kT = [big.tile([128, S], BF16, name=f"kT{f}") for f in range(FT)]
            vb = [big.tile([128, H, HD + 1], BF16, name=f"vb{t}") for t in range(ST)]
            # wd stored per-head ([64, 768] at partition base 0) so the dense
            # per-head K=64 matmuls have base-aligned lhsT/rhs
            dw_bf = [big.tile([HD, HID], BF16, name=f"dwbf{h}") for h in range(H)]

            # ---- projections ----
            def project_chunk(x_d, xT_c, ps_tp, xn_pool, chunk):
                """DMA 512 rows of x, transpose into xT_c [128, FT, 512] f32."""
                for ss in range(4):
                    x_nat = xn_pool.tile([128, HID], F32, name="x_nat")
                    st = chunk * 4 + ss
                    nc.sync.dma_start(
                        out=x_nat, in_=x_d[st * 128 : (st + 1) * 128, :]
                    )
                    for f in range(FT):
                        tp_ps = ps_tp.tile([128, 128], F32, name="tp_ps")
                        nc.tensor.transpose(
                            tp_ps, x_nat[:, f * 128 : (f + 1) * 128], ident
                        )
                        nc.vector.tensor_copy(
                            out=xT_c[:, f, ss * 128 : (ss + 1) * 128], in_=tp_ps
                        )

            with (
                tc.tile_pool(name="wq_pool", bufs=1) as wq_pool,
                tc.tile_pool(name="xn", bufs=3) as xn_pool,
                tc.tile_pool(name="xT", bufs=2) as xT_pool,
                tc.tile_pool(name="ps_tp", bufs=4, space="PSUM") as ps_tp,
                tc.tile_pool(name="ps_pj", bufs=2, space="PSUM") as ps_pj,
            ):
                wq_b = [
                    wq_pool.tile([128, HID], BF16, name=f"wq{f}") for f in range(FT)
                ]
                for f in range(FT):
                    wtmp = xn_pool.tile([128, HID], F32, name="wtmp")
                    nc.sync.dma_start(out=wtmp, in_=wq_d[f * 128 : (f + 1) * 128, :])
                    nc.vector.tensor_copy(out=wq_b[f], in_=wtmp)
                # load wd (fp32) per head and convert to bf16
                for h in range(H):
                    wd_t = xn_pool.tile([HD, HID], F32, name="wd_t")
                    nc.sync.dma_start(out=wd_t, in_=wd_d[h * HD : (h + 1) * HD, :])
                    nc.vector.tensor_copy(out=dw_bf[h], in_=wd_t)

                for chunk in range(QT):
                    xT_c = xT_pool.tile([128, FT, 512], BF16, name="xT_q")
                    project_chunk(xq_d, xT_c, ps_tp, xn_pool, chunk)
                    for fo in range(FT):
                        pj = ps_pj.tile([128, 512], F32, name="pj")
                        for kf in range(FT):
                            nc.tensor.matmul(
                                pj,
                                wq_b[kf][:, fo * 128 : (fo + 1) * 128],
                                xT_c[:, kf, :],
                                start=(kf == 0),
                                stop=(kf == FT - 1),
                            )
                        nc.vector.tensor_scalar_add(
                            out=qT[fo][:, chunk * 512 : (chunk + 1) * 512],
                            in0=pj,
                            scalar1=bqc[:, fo : fo + 1],
                        )

            with (
                tc.tile_pool(name="wkv_pool", bufs=1) as wkv_pool,
                tc.tile_pool(name="xn2", bufs=3) as xn2_pool,
                tc.tile_pool(name="xT2", bufs=2) as xT2_pool,
                tc.tile_pool(name="ps_tp2", bufs=2, space="PSUM") as ps_tp2,
                tc.tile_pool(name="ps_pj2", bufs=2, space="PSUM") as ps_pj2,
                tc.tile_pool(name="ps_v", bufs=2, space="PSUM") as ps_v,
            ):
                wk_b = [
                    wkv_pool.tile([128, HID], BF16, name=f"wk{f}") for f in range(FT)
                ]
                wv_b = [
                    wkv_pool.tile([128, HID], BF16, name=f"wv{f}") for f in range(FT)
                ]
                for f in range(FT):
                    wtmp = xn2_pool.tile([128, HID], F32, name="wtmp2")
                    nc.sync.dma_start(out=wtmp, in_=wk_d[f * 128 : (f + 1) * 128, :])
                    nc.vector.tensor_copy(out=wk_b[f], in_=wtmp)
                    wtmp = xn2_pool.tile([128, HID], F32, name="wtmp2")
                    nc.sync.dma_start(out=wtmp, in_=wv_d[f * 128 : (f + 1) * 128, :])
                    nc.vector.tensor_copy(out=wv_b[f], in_=wtmp)

                for chunk in range(QT):
                    xT_c = xT2_pool.tile([128, FT, 512], BF16, name="xT_kv")
                    project_chunk(xkv_d, xT_c, ps_tp2, xn2_pool, chunk)
                    # kT
                    for fo in range(FT):
                        pj = ps_pj2.tile([128, 512], F32, name="pj2")
                        for kf in range(FT):
                            nc.tensor.matmul(
                                pj,
                                wk_b[kf][:, fo * 128 : (fo + 1) * 128],
                                xT_c[:, kf, :],
                                start=(kf == 0),
                                stop=(kf == FT - 1),
                            )
                        nc.vector.tensor_scalar_add(
                            out=kT[fo][:, chunk * 512 : (chunk + 1) * 512],
                            in0=pj,
                            scalar1=bkc[:, fo : fo + 1],
                        )
                    # v (natural layout, rows scaled by exp(mask), + denom col)
                    for ss in range(4):
                        st = chunk * 4 + ss
                        vp = ps_v.tile([128, NH, 512], F32, name="vp")
                        for nh in range(NH):
                            for kf in range(FT):
                                nc.tensor.matmul(
                                    vp[:, nh, 0:NW],
                                    xT_c[:, kf, ss * 128 : (ss + 1) * 128],
                                    wv_b[kf][:, nh * NW : (nh + 1) * NW],
                                    start=(kf == 0),
                                    stop=False,
                                )
                            nc.tensor.matmul(
                                vp[:, nh, 0:NW],
                                ones_r,
                                bv_row[0:1, nh * NW : (nh + 1) * NW],
                                start=False,
                                stop=True,
                            )
                        emcol = emask[:, st : st + 1]
                        for nh in range(NH):
                            nc.vector.tensor_scalar_mul(
                                out=vb[st][:, nh * 6 : (nh + 1) * 6, 0:HD],
                                in0=vp[:, nh, 0:NW].rearrange(
                                    "p (a d) -> p a d", a=6
                                ),
                                scalar1=emcol,
                            )
                        nc.vector.tensor_scalar_mul(
                            out=vb[st][:, :, HD : HD + 1].rearrange(
                                "p a c -> p (a c)"
                            ),
                            in0=ones_12,
                            scalar1=emcol,
                        )

            # ---- attention + dense + layernorm, per 512-wide q chunk ----
            with (
                tc.tile_pool(name="ctx_pool", bufs=2) as ctx_pool,
                tc.tile_pool(name="exp_pool", bufs=3) as exp_pool,
                tc.tile_pool(name="rec_pool", bufs=2) as rec_pool,
                tc.tile_pool(name="res_pool", bufs=3) as res_pool,
                tc.tile_pool(name="hpre_pool", bufs=2) as hpre_pool,
                tc.tile_pool(name="st_pool", bufs=4) as st_pool,
                tc.tile_pool(name="ps_sc", bufs=2, space="PSUM") as ps_sc,
                tc.tile_pool(name="ps_ctx", bufs=1, space="PSUM") as ps_ctx,
                tc.tile_pool(name="ps_bc", bufs=1, space="PSUM") as ps_bc,
                tc.tile_pool(name="ps_h", bufs=1, space="PSUM") as ps_h,
            ):
                for qt in range(QT):
                    ctx_t = [
                        ctx_pool.tile([HD, 512], BF16, name=f"ctx{h}")
                        for h in range(H)
                    ]
                    qsl = slice(qt * 512, (qt + 1) * 512)
                    for h in range(H):
                        ft, po = h // 2, (h % 2) * 64
                        ctx_ps = ps_ctx.tile([HD + 1, 512], F32, name="ctx_ps")
                        for g in range(8):
                            sc_ps = ps_sc.tile([128, 2, 512], F32, name="sc_ps")
                            for j in range(2):
                                kc = g * 2 + j
                                nc.tensor.matmul(
                                    sc_ps[:, j, :],
                                    kT[ft][po : po + HD, kc * 128 : (kc + 1) * 128],
                                    qT[ft][po : po + HD, qsl],
                                    start=True,
                                    stop=True,
                                )
                            exp_g = exp_pool.tile([128, 2, 512], BF16, name="exp_g")
                            nc.scalar.activation(
                                out=exp_g, in_=sc_ps, func=AF.Exp, scale=0.125
                            )
                            for j in range(2):
                                kc = g * 2 + j
                                # rows 0..63 = unnormalized ctx, row 64 = denom
                                nc.tensor.matmul(
                                    ctx_ps,
                                    vb[kc][:, h, :],
                                    exp_g[:, j, :],
                                    start=(g == 0 and j == 0),
                                    stop=(g == 7 and j == 1),
                                )
                        # recip of denom (partition 64), broadcast to 0..63
                        # via a K=1 matmul whose contraction row is base 64
                        rec = rec_pool.tile([HD + 1, 512], F32, name="rec")
                        nc.vector.reciprocal(
                            rec[HD : HD + 1, :], ctx_ps[HD : HD + 1, :]
                        )
                        rec_b = rec_pool.tile([HD + 1, 512], BF16, name="rec_b")
                        nc.vector.tensor_copy(
                            out=rec_b[HD : HD + 1, :], in_=rec[HD : HD + 1, :]
                        )
                        bc_ps = ps_bc.tile([HD, 512], F32, name="bc_ps")
                        nc.tensor.matmul(
                            bc_ps,
                            ones2[HD : HD + 1, :],
                            rec_b[HD : HD + 1, :],
                            start=True,
                            stop=True,
                        )
                        bc_sb = rec_pool.tile([HD, 512], F32, name="bc_sb")
                        nc.vector.tensor_copy(out=bc_sb, in_=bc_ps)
                        nc.vector.tensor_mul(
                            out=ctx_t[h],
                            in0=ctx_ps[0:HD, :],
                            in1=bc_sb,
                        )

                    for ss in range(4):
                        st = qt * 4 + ss
                        ssl = slice(ss * 128, (ss + 1) * 128)
                        h_ps = ps_h.tile([128, NH, 512], F32, name="h_ps")
                        for nh in range(NH):
                            for h in range(H):
                                nc.tensor.matmul(
                                    h_ps[:, nh, 0:NW],
                                    ctx_t[h][:, ssl],
                                    dw_bf[h][:, nh * NW : (nh + 1) * NW],
                                    start=(h == 0),
                                    stop=False,
                                )
                            nc.tensor.matmul(
                                h_ps[:, nh, 0:NW],
                                ones_r,
                                bd_row[0:1, nh * NW : (nh + 1) * NW],
                                start=False,
                                stop=True,
                            )
                        x_res = res_pool.tile([128, HID], F32, name="x_res")
                        nc.sync.dma_start(
                            out=x_res, in_=xkv_d[st * 128 : (st + 1) * 128, :]
                        )
                        hp = hpre_pool.tile([128, HID], F32, name="hp")
                        nc.vector.tensor_add(
                            out=hp.rearrange("p (a w) -> p a w", a=NH),
                            in0=h_ps[:, :, 0:NW],
                            in1=x_res.rearrange("p (a w) -> p a w", a=NH),
                        )
                        stats = st_pool.tile([128, 3, 6], F32, name="stats")
                        for sg in range(3):
                            nc.vector.bn_stats(
                                out=stats[:, sg, :], in_=hp[:, sg * 256 : (sg + 1) * 256]
                            )
                        mv = st_pool.tile([128, 2], F32, name="mv")
                        nc.vector.bn_aggr(out=mv, in_=stats)
                        # rstd = exp(-0.5*ln(var+eps)); keeps ACT on the
                        # exp/ln table set (no sqrt-set thrash)
                        lnv = st_pool.tile([128, 1], F32, name="lnv")
                        nc.scalar.activation(
                            out=lnv, in_=mv[:, 1:2], func=AF.Ln, bias=eps_t, scale=1.0
                        )
                        rstd = st_pool.tile([128, 1], F32, name="rstd")
                        nc.scalar.activation(
                            out=rstd, in_=lnv, func=AF.Exp, scale=-0.5
                        )
                        hn = hpre_pool.tile([128, HID], F32, name="hn")
                        # (hp - mu) * rstd in one DVE op
                        nc.vector.tensor_scalar(
                            out=hn,
                            in0=hp,
                            scalar1=mv[:, 0:1],
                            scalar2=rstd,
                            op0=mybir.AluOpType.subtract,
                            op1=mybir.AluOpType.mult,
                        )
                        nc.vector.tensor_mul(hn, hn, g_bc)
                        nc.vector.tensor_add(hn, hn, b_bc)
                        nc.sync.dma_start(
                            out=out_d[st * 128 : (st + 1) * 128, :], in_=hn
                        )

    nc.compile()
    return nc


_NC = None


def _get_nc():
    global _NC
    if _NC is None:
        _NC = build_nc()
    return _NC


def _prepare(
    input_tensor1, attention_mask1, input_tensor2, attention_mask2,
    q1_w, q1_b, k1_w, k1_b, v1_w, v1_b,
    q2_w, q2_b, k2_w, k2_b, v2_w, v2_b,
    d1_w, d1_b, d2_w, d2_b, ln1_g, ln1_b, ln2_g, ln2_b,
):
    f = lambda a: np.ascontiguousarray(np.asarray(a), dtype=np.float32)
    x1, x2 = f(input_tensor1), f(input_tensor2)
    m1 = f(attention_mask1).reshape(B, S, 1)
    m2 = f(attention_mask2).reshape(B, S, 1)
    row = lambda a: f(a).reshape(1, HID)

    in_maps = []
    for b in range(B):
        # stream1: ctx1 = attend(q2, k1, v1, mask1); out h1[b]
        in_maps.append({
            "xq": x2[b], "xkv": x1[b],
            "wq": f(q2_w), "wk": f(k1_w), "wv": f(v1_w), "wd": f(d1_w),
            "bq": row(q2_b), "bk": row(k1_b), "bv": row(v1_b), "bd": row(d1_b),
            "mask": m1[b], "lng": row(ln1_g), "lnb": row(ln1_b),
        })
    for b in range(B):
        # stream2: ctx2 = attend(q1, k2, v2, mask2); out h2[b]
        in_maps.append({
            "xq": x1[b], "xkv": x2[b],
            "wq": f(q1_w), "wk": f(k2_w), "wv": f(v2_w), "wd": f(d2_w),
            "bq": row(q1_b), "bk": row(k2_b), "bv": row(v2_b), "bd": row(d2_b),
            "mask": m2[b], "lng": row(ln2_g), "lnb": row(ln2_b),
        })

    return in_maps


def _run(in_maps, **kwargs):
    nc = _get_nc()
    res = bass_utils.run_bass_kernel_spmd(
        nc, in_maps, core_ids=list(range(8)), **kwargs
    )
    h1 = np.stack([res.results[b]["out"] for b in range(B)])
    h2 = np.stack([res.results[B + b]["out"] for b in range(B)])
    return (h1, h2), res


def kernel(**inputs):
    (h1, h2), _ = _run(_prepare(**inputs))
    return h1, h2
